# revision 22
# baseline (speedup 1.0000x reference)
"""Bass/Trainium2 kernel for BiasedMultiheadAttention.

Problem shapes (hardcoded): B=2, L=2048, D=1024, H=16, d=64.
Sharding: 8 cores = 2 batches x 4 head-groups (4 heads per core).
Each core computes its heads' attention and a partial out-projection;
host sums the 4 partials per batch and adds b_out.

Device dataflow per core (b, heads h0..h0+3):
  in-proj  : qT,kT ([d,L] layout, bf16, q pre-scaled 1/sqrt(d)) and v ([L,d] layout)
  QK^T     : S^T[k,q] psum tiles, 2-head row-tiled (K=64 each at rows 0-63/64-127)
  bias     : DVE adds attn_bias^T tile in-place in PSUM
  softmax  : ACT exp with per-partition bias = -1e4*pad[k] (pad folds in free);
             no max-subtraction needed (scores bounded; exp(-1e4)->0 matches the
             reference's clamp semantics to ~2e-9 absolute in attn weights)
  Z        : ones-matmul column sums (M=1)
  AV       : O^T[dv,q] psum, 2-head col-tiled at (0,0)/(0,64), separate banks
  norm     : reciprocal(Z) -> K=1 ones-matmul broadcast -> DVE multiply
  out-proj : partial[l,j] over this core's 256 head-dims, fp32 to DRAM
"""

import numpy as np
import ml_dtypes

B, L, D, H = 2, 2048, 1024, 16
NHC = 4          # heads per core
d = 64
QB = 512         # query block (matmul moving free dim)

_BF16 = ml_dtypes.bfloat16

_cached = {}


def _build_nc(Lx=L, loop_n=1, id4=4):
    import contextlib

    import concourse.bacc as bacc
    import concourse.mybir as mybir
    import concourse.tile as tile

    fp32 = mybir.dt.float32
    bf16 = mybir.dt.bfloat16
    Ident = mybir.ActivationFunctionType.Identity
    Exp = mybir.ActivationFunctionType.Exp

    nqb = Lx // QB
    nkc = Lx // 128
    nlc = Lx // 128

    nc = bacc.Bacc("TRN2", target_bir_lowering=False)

    xT_d = nc.dram_tensor("xT", [D, Lx], bf16, kind="ExternalInput")
    wqkT_d = nc.dram_tensor("wqkT", [D, 512], bf16, kind="ExternalInput")
    wvT_d = nc.dram_tensor("wvT", [D, 256], bf16, kind="ExternalInput")
    woT_d = nc.dram_tensor("woT", [256, D], bf16, kind="ExternalInput")
    bqk_d = nc.dram_tensor("bqk", [128, 4], fp32, kind="ExternalInput")
    bvr_d = nc.dram_tensor("bvr", [128, 256], fp32, kind="ExternalInput")
    biasT_d = nc.dram_tensor("biasT", [NHC, Lx, Lx], bf16, kind="ExternalInput")
    padj_d = nc.dram_tensor("padj", [Lx], fp32, kind="ExternalInput")
    ident_d = nc.dram_tensor("ident", [128, 128], bf16, kind="ExternalInput")
    out_d = nc.dram_tensor("partial", [Lx, D], fp32, kind="ExternalOutput")

    with tile.TileContext(nc) as tc:
        with contextlib.ExitStack() as ctx:
            const = ctx.enter_context(tc.tile_pool(name="const", bufs=1))
            biasp = ctx.enter_context(tc.tile_pool(name="biasp", bufs=20))
            probsp = ctx.enter_context(tc.tile_pool(name="probsp", bufs=6))
            outp = ctx.enter_context(tc.tile_pool(name="outp", bufs=3))
            zrecp = ctx.enter_context(tc.tile_pool(name="zrecp", bufs=2))
            # 8 PSUM banks: 2x2 (qk pair tiles) + 2 (ot) + 2 (misc)
            ps_qk = ctx.enter_context(tc.tile_pool(name="ps_qk", bufs=2, space="PSUM"))
            ps_ot = ctx.enter_context(tc.tile_pool(name="ps_ot", bufs=2, space="PSUM"))
            ps_misc = ctx.enter_context(
                tc.tile_pool(name="ps_misc", bufs=2, space="PSUM")
            )

            def _emit():
                # ---- persistent SBUF ----
                xT_sb = const.tile([128, 8, Lx], bf16, name="xT_sb", tag="xT_sb")
                nc.sync.dma_start(
                    xT_sb[:], xT_d.rearrange("(dc p) l -> p dc l", p=128)
                )
                wqkT_sb = const.tile([128, 8, 512], bf16, name="wqkT_sb", tag="wqkT_sb")
                nc.sync.dma_start(
                    wqkT_sb[:], wqkT_d.rearrange("(dc p) f -> p dc f", p=128)
                )
                wvT_sb = const.tile([128, 8, 256], bf16, name="wvT_sb", tag="wvT_sb")
                nc.sync.dma_start(
                    wvT_sb[:], wvT_d.rearrange("(dc p) f -> p dc f", p=128)
                )
                woT_sb2 = const.tile([64, 4, D], bf16, name="woT_sb2", tag="woT_sb2")
                nc.sync.dma_start(
                    woT_sb2[:], woT_d.rearrange("(hh p) j -> p hh j", p=64)
                )
                bqk_sb = const.tile([128, 4], fp32, name="bqk_sb", tag="bqk_sb")
                nc.sync.dma_start(bqk_sb[:], bqk_d[:])
                bvr_sb = const.tile([128, 256], fp32, name="bvr_sb", tag="bvr_sb")
                nc.sync.dma_start(bvr_sb[:], bvr_d[:])
                padj_sb = const.tile([128, nkc], fp32, name="padj_sb", tag="padj_sb")
                nc.sync.dma_start(padj_sb[:], padj_d.rearrange("(kc p) -> p kc", p=128))

                qkT_sb = const.tile([128, 4, Lx], bf16, name="qkT_sb", tag="qkT_sb")
                v_sb = const.tile([128, nlc, 4, 65], bf16, name="v_sb", tag="v_sb")
                nc.vector.memset(v_sb[:, :, :, 64:65], 1.0)
                otn_sb = const.tile([64, 4, Lx], bf16, name="otn_sb", tag="otn_sb")
                ones_f32 = const.tile([65, 64], fp32, name="ones_f32", tag="ones_f32")
                nc.vector.memset(ones_f32[:], 1.0)
                ident_sb = const.tile([128, 128], bf16, name="ident_sb", tag="ident_sb")
                nc.sync.dma_start(ident_sb[:], ident_d[:])

                # ---- in-projection: qT / kT (features on partitions) ----
                for m in range(4):
                    for nb in range(nqb):
                        ps = ps_misc.tile(
                            [128, QB], fp32, name="ps_iqk", tag="ps_misc"
                        )
                        for dc in range(8):
                            nc.tensor.matmul(
                                ps[:],
                                wqkT_sb[:, dc, m * 128 : (m + 1) * 128],
                                xT_sb[:, dc, nb * QB : (nb + 1) * QB],
                                start=(dc == 0),
                                stop=(dc == 7),
                            )
                        nc.scalar.activation(
                            qkT_sb[:, m, nb * QB : (nb + 1) * QB],
                            ps[:],
                            Ident,
                            bias=bqk_sb[:, m : m + 1],
                            scale=0.125 if m < 2 else 1.0,
                        )

                # ---- in-projection: v (tokens on partitions) ----
                for lc in range(nlc):
                    ps = ps_misc.tile([128, QB], fp32, name="ps_iv", tag="ps_misc")
                    psv = ps[:, :256]
                    for dc in range(8):
                        nc.tensor.matmul(
                            psv,
                            xT_sb[:, dc, lc * 128 : (lc + 1) * 128],
                            wvT_sb[:, dc, :],
                            start=(dc == 0),
                            stop=(dc == 7),
                        )
                    nc.vector.tensor_add(
                        v_sb[:, lc, :, 0:64],
                        psv.rearrange("p (h x) -> p h x", h=4),
                        bvr_sb.rearrange("p (h x) -> p h x", h=4),
                    )

                # ---- attention ----
                for qb in range(nqb):
                    for hp in range(2):
                        # one bank per head; row 64 of each accumulates Z
                        # (ones-column in v), rows 0-63 accumulate O^T
                        ot_a = ps_ot.tile([65, QB], fp32, name="ot_a", tag="ps_ot")
                        ot_b = ps_ot.tile([65, QB], fp32, name="ot_b", tag="ps_ot")
                        for kc in range(nkc):
                            sp = ps_qk.tile(
                                [128, 2 * QB], fp32, name="sp", tag="ps_qk"
                            )
                            btab = biasp.tile(
                                [128, 2 * QB], bf16, name="btab", tag="bias"
                            )
                            nc.sync.dma_start(
                                btab[:, 0:QB],
                                biasT_d[
                                    2 * hp,
                                    kc * 128 : (kc + 1) * 128,
                                    qb * QB : (qb + 1) * QB,
                                ],
                            )
                            nc.sync.dma_start(
                                btab[:, QB : 2 * QB],
                                biasT_d[
                                    2 * hp + 1,
                                    kc * 128 : (kc + 1) * 128,
                                    qb * QB : (qb + 1) * QB,
                                ],
                            )
                            use_id = (kc % 4) < id4
                            if use_id:
                                # preload bias into PSUM via identity matmul,
                                # QK accumulates on top
                                nc.tensor.matmul(
                                    sp[:, 0:QB],
                                    ident_sb[:],
                                    btab[:, 0:QB],
                                    start=True,
                                    stop=False,
                                )
                                nc.tensor.matmul(
                                    sp[:, QB : 2 * QB],
                                    ident_sb[:],
                                    btab[:, QB : 2 * QB],
                                    start=True,
                                    stop=False,
                                )
                            nc.tensor.matmul(
                                sp[:, 0:QB],
                                qkT_sb[0:64, 2 + hp, kc * 128 : (kc + 1) * 128],
                                qkT_sb[0:64, hp, qb * QB : (qb + 1) * QB],
                                start=not use_id,
                                stop=True,
                            )
                            nc.tensor.matmul(
                                sp[:, QB : 2 * QB],
                                qkT_sb[64:128, 2 + hp, kc * 128 : (kc + 1) * 128],
                                qkT_sb[64:128, hp, qb * QB : (qb + 1) * QB],
                                start=not use_id,
                                stop=True,
                            )
                            if not use_id:
                                nc.vector.tensor_add(sp[:], sp[:], btab[:])
                            prob = probsp.tile(
                                [128, 2 * QB], bf16, name="prob", tag="probs"
                            )
                            nc.scalar.activation(
                                prob[:], sp[:], Exp, bias=padj_sb[:, kc : kc + 1]
                            )
                            proba = prob[:, 0:QB]
                            probb = prob[:, QB : 2 * QB]
                            # O^T (+Z in row 64) accumulation, aug-v lhsT M=65
                            nc.tensor.matmul(
                                ot_a[:, :],
                                v_sb[:, kc, 2 * hp, :],
                                proba,
                                start=(kc == 0),
                                stop=(kc == nkc - 1),
                            )
                            nc.tensor.matmul(
                                ot_b[:, :],
                                v_sb[:, kc, 2 * hp + 1, :],
                                probb,
                                start=(kc == 0),
                                stop=(kc == nkc - 1),
                            )
                        # normalize: Z sits in row 64 of each ot bank
                        zrec = zrecp.tile([65, 2 * QB], fp32, name="zrec", tag="zrec")
                        nc.vector.reciprocal(zrec[64:65, 0:QB], ot_a[64:65, :])
                        nc.vector.reciprocal(zrec[64:65, QB : 2 * QB], ot_b[64:65, :])
                        zb_a = ps_misc.tile([64, QB], fp32, name="zb_a", tag="ps_misc")
                        zb_b = ps_misc.tile([64, QB], fp32, name="zb_b", tag="ps_misc")
                        nc.tensor.matmul(
                            zb_a[0:64, :],
                            ones_f32[64:65, :],
                            zrec[64:65, 0:QB],
                            start=True,
                            stop=True,
                            tile_position=(64, 0),
                        )
                        nc.tensor.matmul(
                            zb_b[0:64, :],
                            ones_f32[64:65, :],
                            zrec[64:65, QB : 2 * QB],
                            start=True,
                            stop=True,
                            tile_position=(64, 0),
                        )
                        zb_sb = zrecp.tile(
                            [64, 2 * QB], fp32, name="zb_sb", tag="zb_sb"
                        )
                        nc.vector.tensor_copy(zb_sb[:, 0:QB], zb_a[:, :])
                        nc.vector.tensor_copy(zb_sb[:, QB : 2 * QB], zb_b[:, :])
                        nc.vector.tensor_mul(
                            otn_sb[:, 2 * hp, qb * QB : (qb + 1) * QB],
                            ot_a[0:64, :],
                            zb_sb[:, 0:QB],
                        )
                        nc.vector.tensor_mul(
                            otn_sb[:, 2 * hp + 1, qb * QB : (qb + 1) * QB],
                            ot_b[0:64, :],
                            zb_sb[:, QB : 2 * QB],
                        )

                    # ---- partial out-projection for this query block ----
                    for lc in range(qb * (QB // 128), (qb + 1) * (QB // 128)):
                        for jb in range(2):
                            pps = ps_misc.tile(
                                [128, QB], fp32, name="pps", tag="ps_misc"
                            )
                            for hh in range(4):
                                nc.tensor.matmul(
                                    pps[:],
                                    otn_sb[:, hh, lc * 128 : (lc + 1) * 128],
                                    woT_sb2[
                                        0:64, hh, jb * QB : (jb + 1) * QB
                                    ],
                                    start=(hh == 0),
                                    stop=(hh == 3),
                                )
                            osb = outp.tile([128, QB], fp32, name="osb", tag="osb")
                            nc.vector.tensor_copy(osb[:], pps[:])
                            nc.sync.dma_start(
                                out_d[
                                    lc * 128 : (lc + 1) * 128,
                                    jb * QB : (jb + 1) * QB,
                                ],
                                osb[:],
                            )

            if loop_n <= 1:
                _emit()
            else:
                with tc.For_i(0, loop_n, 1):
                    _emit()

    nc.compile()
    return nc


def _shard_inputs(x, key_padding_mask, attn_bias, W_in, b_in, W_out, b_out, Lx=L):
    """Host-side layout prep: slice per core, transpose/cast. No math beyond
    bias folding (b_q/8, -1e4*pad)."""
    in_maps = []
    W_out_T = np.ascontiguousarray(W_out.T)
    for c in range(8):
        b = c // 4
        h0 = (c % 4) * NHC
        rows_q = slice(h0 * d, (h0 + NHC) * d)
        rows_k = slice(D + h0 * d, D + (h0 + NHC) * d)
        rows_v = slice(2 * D + h0 * d, 2 * D + (h0 + NHC) * d)
        wqk = np.concatenate([W_in[rows_q], W_in[rows_k]], axis=0)  # [512, D]
        wqkT = np.ascontiguousarray(wqk.T).astype(_BF16)
        wvT = np.ascontiguousarray(W_in[rows_v].T).astype(_BF16)
        woT = np.ascontiguousarray(W_out_T[rows_q]).astype(_BF16)
        bqk_vec = np.concatenate([b_in[rows_q] / 8.0, b_in[rows_k]]).astype(np.float32)
        bqk = np.ascontiguousarray(bqk_vec.reshape(4, 128).T)
        bvr = np.ascontiguousarray(
            np.broadcast_to(b_in[rows_v].astype(np.float32), (128, 256))
        )
        biasT = np.ascontiguousarray(
            attn_bias[b, h0 : h0 + NHC].transpose(0, 2, 1)
        ).astype(_BF16)
        padj = (-10000.0 * key_padding_mask[b]).astype(np.float32)
        xT = np.ascontiguousarray(x[b].T).astype(_BF16)
        in_maps.append(
            {
                "xT": xT,
                "wqkT": wqkT,
                "wvT": wvT,
                "woT": woT,
                "bqk": bqk,
                "bvr": bvr,
                "biasT": biasT,
                "padj": padj,
                "ident": np.eye(128, dtype=_BF16),
            }
        )
    return in_maps


def kernel(x, key_padding_mask, attn_bias, W_in, b_in, W_out, b_out):
    from concourse.bass_utils import run_bass_kernel_spmd

    if "nc" not in _cached:
        _cached["nc"] = _build_nc()
    nc = _cached["nc"]

    in_maps = _shard_inputs(
        np.asarray(x),
        np.asarray(key_padding_mask),
        np.asarray(attn_bias),
        np.asarray(W_in),
        np.asarray(b_in),
        np.asarray(W_out),
        np.asarray(b_out),
    )
    res = run_bass_kernel_spmd(nc, in_maps, core_ids=list(range(8)))
    out = np.empty((B, L, D), dtype=np.float32)
    b_out32 = np.asarray(b_out).astype(np.float32)
    for b in range(B):
        acc = res.results[4 * b]["partial"].astype(np.float32).copy()
        for c in range(4 * b + 1, 4 * b + 4):
            acc += res.results[c]["partial"]
        out[b] = acc + b_out32
    return out


# revision 25
# speedup vs baseline: 1.0759x; 1.0759x over previous
"""Bass/Trainium2 kernel for BiasedMultiheadAttention.

Problem shapes (hardcoded): B=2, L=2048, D=1024, H=16, d=64.
Sharding: 8 cores = 2 batches x 4 head-groups (4 heads per core).
Each core computes its heads' attention and a partial out-projection;
host sums the 4 partials per batch and adds b_out.

Device dataflow per core (b, heads h0..h0+3):
  in-proj  : qT,kT ([d,L] layout, bf16, q pre-scaled 1/sqrt(d)) and v ([L,d] layout)
  QK^T     : S^T[k,q] psum tiles, 2-head row-tiled (K=64 each at rows 0-63/64-127)
  bias     : DVE adds attn_bias^T tile in-place in PSUM
  softmax  : ACT exp with per-partition bias = -1e4*pad[k] (pad folds in free);
             no max-subtraction needed (scores bounded; exp(-1e4)->0 matches the
             reference's clamp semantics to ~2e-9 absolute in attn weights)
  Z        : ones-matmul column sums (M=1)
  AV       : O^T[dv,q] psum, 2-head col-tiled at (0,0)/(0,64), separate banks
  norm     : reciprocal(Z) -> K=1 ones-matmul broadcast -> DVE multiply
  out-proj : partial[l,j] over this core's 256 head-dims, fp32 to DRAM
"""

import numpy as np
import ml_dtypes

B, L, D, H = 2, 2048, 1024, 16
NHC = 4          # heads per core
d = 64
QB = 512         # query block (matmul moving free dim)

_BF16 = ml_dtypes.bfloat16

_cached = {}


def _build_nc(Lx=L, loop_n=1, id4=4):
    import contextlib

    import concourse.bacc as bacc
    import concourse.mybir as mybir
    import concourse.tile as tile

    fp32 = mybir.dt.float32
    bf16 = mybir.dt.bfloat16
    Ident = mybir.ActivationFunctionType.Identity
    Exp = mybir.ActivationFunctionType.Exp

    nqb = Lx // QB
    nkc = Lx // 128
    nlc = Lx // 128

    nc = bacc.Bacc("TRN2", target_bir_lowering=False)

    xT_d = nc.dram_tensor("xT", [D, Lx], bf16, kind="ExternalInput")
    wqkT_d = nc.dram_tensor("wqkT", [D, 512], bf16, kind="ExternalInput")
    wvT_d = nc.dram_tensor("wvT", [D, 256], bf16, kind="ExternalInput")
    woT_d = nc.dram_tensor("woT", [256, D], bf16, kind="ExternalInput")
    bqk_d = nc.dram_tensor("bqk", [128, 4], fp32, kind="ExternalInput")
    bvr_d = nc.dram_tensor("bvr", [128, 256], fp32, kind="ExternalInput")
    biasT_d = nc.dram_tensor("biasT", [2, Lx // 128, Lx // QB, 128, 2 * QB], bf16, kind="ExternalInput")
    padj_d = nc.dram_tensor("padj", [Lx], fp32, kind="ExternalInput")
    ident_d = nc.dram_tensor("ident", [128, 128], bf16, kind="ExternalInput")
    out_d = nc.dram_tensor("partial", [Lx, D], fp32, kind="ExternalOutput")

    with tile.TileContext(nc) as tc:
        with contextlib.ExitStack() as ctx:
            const = ctx.enter_context(tc.tile_pool(name="const", bufs=1))
            biasp = ctx.enter_context(tc.tile_pool(name="biasp", bufs=20))
            probsp = ctx.enter_context(tc.tile_pool(name="probsp", bufs=6))
            outp = ctx.enter_context(tc.tile_pool(name="outp", bufs=3))
            zrecp = ctx.enter_context(tc.tile_pool(name="zrecp", bufs=2))
            # 8 PSUM banks: 3x2 (big shared slots) + 2x1 (ot)
            ps_qk = ctx.enter_context(tc.tile_pool(name="ps_qk", bufs=3, space="PSUM"))
            ps_ot = ctx.enter_context(tc.tile_pool(name="ps_ot", bufs=2, space="PSUM"))
            ps_misc = ps_qk

            def _emit():
                # ---- persistent SBUF ----
                xT_sb = const.tile([128, 8, Lx], bf16, name="xT_sb", tag="xT_sb")
                xT_r = xT_d.rearrange("(dc p) l -> p dc l", p=128)
                for dc in range(8):
                    nc.sync.dma_start(xT_sb[:, dc, :], xT_r[:, dc, :])
                wqkT_sb = const.tile([128, 8, 512], bf16, name="wqkT_sb", tag="wqkT_sb")
                nc.sync.dma_start(
                    wqkT_sb[:], wqkT_d.rearrange("(dc p) f -> p dc f", p=128)
                )
                wvT_sb = const.tile([128, 8, 256], bf16, name="wvT_sb", tag="wvT_sb")
                nc.sync.dma_start(
                    wvT_sb[:], wvT_d.rearrange("(dc p) f -> p dc f", p=128)
                )
                woT_sb2 = const.tile([64, 4, D], bf16, name="woT_sb2", tag="woT_sb2")
                nc.sync.dma_start(
                    woT_sb2[:], woT_d.rearrange("(hh p) j -> p hh j", p=64)
                )
                bqk_sb = const.tile([128, 4], fp32, name="bqk_sb", tag="bqk_sb")
                nc.sync.dma_start(bqk_sb[:], bqk_d[:])
                bvr_sb = const.tile([128, 256], fp32, name="bvr_sb", tag="bvr_sb")
                nc.sync.dma_start(bvr_sb[:], bvr_d[:])
                padj_sb = const.tile([128, nkc], fp32, name="padj_sb", tag="padj_sb")
                nc.sync.dma_start(padj_sb[:], padj_d.rearrange("(kc p) -> p kc", p=128))

                qkT_sb = const.tile([128, 4, Lx], bf16, name="qkT_sb", tag="qkT_sb")
                v_sb = const.tile([128, nlc, 4, 65], bf16, name="v_sb", tag="v_sb")
                nc.vector.memset(v_sb[:, :, :, 64:65], 1.0)
                otn_sb = const.tile([64, 4, Lx], bf16, name="otn_sb", tag="otn_sb")
                ones_f32 = const.tile([65, 64], fp32, name="ones_f32", tag="ones_f32")
                nc.vector.memset(ones_f32[:], 1.0)
                ident_sb = const.tile([128, 128], bf16, name="ident_sb", tag="ident_sb")
                nc.sync.dma_start(ident_sb[:], ident_d[:])

                # ---- in-projection: qT / kT (features on partitions) ----
                for m in range(4):
                    for nb in range(nqb):
                        ps = ps_misc.tile(
                            [128, 2 * QB], fp32, name="ps_iqk", tag="ps_qk"
                        )[:, 0:QB]
                        for dc in range(8):
                            nc.tensor.matmul(
                                ps[:],
                                wqkT_sb[:, dc, m * 128 : (m + 1) * 128],
                                xT_sb[:, dc, nb * QB : (nb + 1) * QB],
                                start=(dc == 0),
                                stop=(dc == 7),
                            )
                        nc.scalar.activation(
                            qkT_sb[:, m, nb * QB : (nb + 1) * QB],
                            ps[:],
                            Ident,
                            bias=bqk_sb[:, m : m + 1],
                            scale=0.125 if m < 2 else 1.0,
                        )

                # ---- in-projection: v (tokens on partitions) ----
                for lc in range(nlc):
                    ps = ps_misc.tile([128, 2 * QB], fp32, name="ps_iv", tag="ps_qk")
                    psv = ps[:, :256]
                    for dc in range(8):
                        nc.tensor.matmul(
                            psv,
                            xT_sb[:, dc, lc * 128 : (lc + 1) * 128],
                            wvT_sb[:, dc, :],
                            start=(dc == 0),
                            stop=(dc == 7),
                        )
                    nc.vector.tensor_add(
                        v_sb[:, lc, :, 0:64],
                        psv.rearrange("p (h x) -> p h x", h=4),
                        bvr_sb.rearrange("p (h x) -> p h x", h=4),
                    )

                # ---- attention ----
                for qb in range(nqb):
                    for hp in range(2):
                        # one bank per head; row 64 of each accumulates Z
                        # (ones-column in v), rows 0-63 accumulate O^T
                        ot_a = ps_ot.tile([65, QB], fp32, name="ot_a", tag="ps_ot")
                        ot_b = ps_ot.tile([65, QB], fp32, name="ot_b", tag="ps_ot")
                        for kc in range(nkc):
                            sp = ps_qk.tile(
                                [128, 2 * QB], fp32, name="sp", tag="ps_qk"
                            )
                            btab = biasp.tile(
                                [128, 2 * QB], bf16, name="btab", tag="bias"
                            )
                            nc.sync.dma_start(btab[:], biasT_d[hp, kc, qb])
                            use_id = (kc % 4) < id4
                            if use_id:
                                # preload bias into PSUM via identity matmul,
                                # QK accumulates on top
                                nc.tensor.matmul(
                                    sp[:, 0:QB],
                                    ident_sb[:],
                                    btab[:, 0:QB],
                                    start=True,
                                    stop=False,
                                )
                                nc.tensor.matmul(
                                    sp[:, QB : 2 * QB],
                                    ident_sb[:],
                                    btab[:, QB : 2 * QB],
                                    start=True,
                                    stop=False,
                                )
                            nc.tensor.matmul(
                                sp[:, 0:QB],
                                qkT_sb[0:64, 2 + hp, kc * 128 : (kc + 1) * 128],
                                qkT_sb[0:64, hp, qb * QB : (qb + 1) * QB],
                                start=not use_id,
                                stop=True,
                            )
                            nc.tensor.matmul(
                                sp[:, QB : 2 * QB],
                                qkT_sb[64:128, 2 + hp, kc * 128 : (kc + 1) * 128],
                                qkT_sb[64:128, hp, qb * QB : (qb + 1) * QB],
                                start=not use_id,
                                stop=True,
                            )
                            if not use_id:
                                nc.vector.tensor_add(sp[:], sp[:], btab[:])
                            prob = probsp.tile(
                                [128, 2 * QB], bf16, name="prob", tag="probs"
                            )
                            nc.scalar.activation(
                                prob[:], sp[:], Exp, bias=padj_sb[:, kc : kc + 1]
                            )
                            proba = prob[:, 0:QB]
                            probb = prob[:, QB : 2 * QB]
                            # O^T (+Z in row 64) accumulation, aug-v lhsT M=65
                            nc.tensor.matmul(
                                ot_a[:, :],
                                v_sb[:, kc, 2 * hp, :],
                                proba,
                                start=(kc == 0),
                                stop=(kc == nkc - 1),
                            )
                            nc.tensor.matmul(
                                ot_b[:, :],
                                v_sb[:, kc, 2 * hp + 1, :],
                                probb,
                                start=(kc == 0),
                                stop=(kc == nkc - 1),
                            )
                        # normalize: Z sits in row 64 of each ot bank
                        zrec = zrecp.tile([65, 2 * QB], fp32, name="zrec", tag="zrec")
                        nc.vector.reciprocal(zrec[64:65, 0:QB], ot_a[64:65, :])
                        nc.vector.reciprocal(zrec[64:65, QB : 2 * QB], ot_b[64:65, :])
                        zb_a = ps_misc.tile(
                            [128, 2 * QB], fp32, name="zb_a", tag="ps_qk"
                        )[0:64, 0:QB]
                        zb_b = ps_misc.tile(
                            [128, 2 * QB], fp32, name="zb_b", tag="ps_qk"
                        )[0:64, 0:QB]
                        nc.tensor.matmul(
                            zb_a[0:64, :],
                            ones_f32[64:65, :],
                            zrec[64:65, 0:QB],
                            start=True,
                            stop=True,
                            tile_position=(64, 0),
                        )
                        nc.tensor.matmul(
                            zb_b[0:64, :],
                            ones_f32[64:65, :],
                            zrec[64:65, QB : 2 * QB],
                            start=True,
                            stop=True,
                            tile_position=(64, 0),
                        )
                        zb_sb = zrecp.tile(
                            [64, 2 * QB], fp32, name="zb_sb", tag="zb_sb"
                        )
                        nc.vector.tensor_copy(zb_sb[:, 0:QB], zb_a[:, :])
                        nc.vector.tensor_copy(zb_sb[:, QB : 2 * QB], zb_b[:, :])
                        nc.vector.tensor_mul(
                            otn_sb[:, 2 * hp, qb * QB : (qb + 1) * QB],
                            ot_a[0:64, :],
                            zb_sb[:, 0:QB],
                        )
                        nc.vector.tensor_mul(
                            otn_sb[:, 2 * hp + 1, qb * QB : (qb + 1) * QB],
                            ot_b[0:64, :],
                            zb_sb[:, QB : 2 * QB],
                        )

                    # ---- partial out-projection for this query block ----
                    for lc in range(qb * (QB // 128), (qb + 1) * (QB // 128)):
                        for jb in range(2):
                            pps = ps_misc.tile(
                                [128, 2 * QB], fp32, name="pps", tag="ps_qk"
                            )[:, 0:QB]
                            for hh in range(4):
                                nc.tensor.matmul(
                                    pps[:],
                                    otn_sb[:, hh, lc * 128 : (lc + 1) * 128],
                                    woT_sb2[
                                        0:64, hh, jb * QB : (jb + 1) * QB
                                    ],
                                    start=(hh == 0),
                                    stop=(hh == 3),
                                )
                            osb = outp.tile([128, QB], fp32, name="osb", tag="osb")
                            nc.vector.tensor_copy(osb[:], pps[:])
                            nc.sync.dma_start(
                                out_d[
                                    lc * 128 : (lc + 1) * 128,
                                    jb * QB : (jb + 1) * QB,
                                ],
                                osb[:],
                            )

            if loop_n <= 1:
                _emit()
            else:
                with tc.For_i(0, loop_n, 1):
                    _emit()

    nc.compile()
    return nc


def _tile_bias(bias4, Lx=L):
    """[4, Lq, Lk] -> tiled bf16 [2, nkc, nqb, 128, 1024]:
    [...,:512] = head 2hp (S^T layout: k on partitions), [...,512:] = head 2hp+1."""
    nkc, nqb = Lx // 128, Lx // QB
    bT = bias4.transpose(0, 2, 1).reshape(4, nkc, 128, nqb, QB)
    # [h, kc, p, qb, q] -> [hp, kc, qb, p, ab, q]
    out = np.empty((2, nkc, nqb, 128, 2 * QB), dtype=_BF16)
    for hp in range(2):
        out[hp, :, :, :, 0:QB] = bT[2 * hp].transpose(0, 2, 1, 3).astype(_BF16)
        out[hp, :, :, :, QB:] = bT[2 * hp + 1].transpose(0, 2, 1, 3).astype(_BF16)
    return out


def _shard_inputs(x, key_padding_mask, attn_bias, W_in, b_in, W_out, b_out, Lx=L):
    """Host-side layout prep: slice per core, transpose/cast. No math beyond
    bias folding (b_q/8, -1e4*pad)."""
    in_maps = []
    W_out_T = np.ascontiguousarray(W_out.T)
    for c in range(8):
        b = c // 4
        h0 = (c % 4) * NHC
        rows_q = slice(h0 * d, (h0 + NHC) * d)
        rows_k = slice(D + h0 * d, D + (h0 + NHC) * d)
        rows_v = slice(2 * D + h0 * d, 2 * D + (h0 + NHC) * d)
        wqk = np.concatenate([W_in[rows_q], W_in[rows_k]], axis=0)  # [512, D]
        wqkT = np.ascontiguousarray(wqk.T).astype(_BF16)
        wvT = np.ascontiguousarray(W_in[rows_v].T).astype(_BF16)
        woT = np.ascontiguousarray(W_out_T[rows_q]).astype(_BF16)
        bqk_vec = np.concatenate([b_in[rows_q] / 8.0, b_in[rows_k]]).astype(np.float32)
        bqk = np.ascontiguousarray(bqk_vec.reshape(4, 128).T)
        bvr = np.ascontiguousarray(
            np.broadcast_to(b_in[rows_v].astype(np.float32), (128, 256))
        )
        biasT = _tile_bias(attn_bias[b, h0 : h0 + NHC])
        padj = (-10000.0 * key_padding_mask[b]).astype(np.float32)
        xT = np.ascontiguousarray(x[b].T).astype(_BF16)
        in_maps.append(
            {
                "xT": xT,
                "wqkT": wqkT,
                "wvT": wvT,
                "woT": woT,
                "bqk": bqk,
                "bvr": bvr,
                "biasT": biasT,
                "padj": padj,
                "ident": np.eye(128, dtype=_BF16),
            }
        )
    return in_maps


def kernel(x, key_padding_mask, attn_bias, W_in, b_in, W_out, b_out):
    from concourse.bass_utils import run_bass_kernel_spmd

    if "nc" not in _cached:
        _cached["nc"] = _build_nc()
    nc = _cached["nc"]

    in_maps = _shard_inputs(
        np.asarray(x),
        np.asarray(key_padding_mask),
        np.asarray(attn_bias),
        np.asarray(W_in),
        np.asarray(b_in),
        np.asarray(W_out),
        np.asarray(b_out),
    )
    res = run_bass_kernel_spmd(nc, in_maps, core_ids=list(range(8)))
    out = np.empty((B, L, D), dtype=np.float32)
    b_out32 = np.asarray(b_out).astype(np.float32)
    for b in range(B):
        acc = res.results[4 * b]["partial"].astype(np.float32).copy()
        for c in range(4 * b + 1, 4 * b + 4):
            acc += res.results[c]["partial"]
        out[b] = acc + b_out32
    return out
